# revision 2
# baseline (speedup 1.0000x reference)
"""DifferentialSoftplus Trainium2 kernel.

Computes, for x[2048,512] f32 and dxdq[2048,512,128] f32:
    out_x    = softplus(x)
    out_dxdq = sigmoid(x)[..., None] * dxdq

Sharding: data-parallel over the leading batch dim across 8 NeuronCores
(256 batch rows per core). Inside each core the (b, n) rows are laid out
so SBUF partition p owns a contiguous chunk of rows: the per-core shard
x[256,512] is host-reshaped to [128, 1024] and dxdq[256,512,128] to
[128, 1024, 128] (pure row-major reshape, no data movement beyond the
device DMA itself). Every DMA is then 128 x contiguous-16KB descriptors
and the per-row sigmoid scalar lands on the right partition with zero
transposes.
"""

import numpy as np

P = 128          # SBUF partitions
B, N, Q = 2048, 512, 128
NCORES = 8
ROWS_PER_CORE = (B * N) // NCORES      # 131072 (b,n) rows per core
TPC = ROWS_PER_CORE // P               # 1024 rows per partition
T = 32                                 # rows per chunk tile -> [128, T*128] f32 = 2 MiB

_cache = {}


def _build_nc(tpc=TPC, t_chunk=T, data_bufs=4):
    import concourse.tile as tile
    from concourse import bacc, mybir

    f32 = mybir.dt.float32
    act = mybir.ActivationFunctionType
    nchunk = tpc // t_chunk

    nc = bacc.Bacc(
        "TRN2", target_bir_lowering=False, debug=False, num_devices=NCORES
    )
    x_d = nc.dram_tensor("x", [P, tpc], f32, kind="ExternalInput").ap()
    dxdq_d = nc.dram_tensor("dxdq", [P, tpc, Q], f32, kind="ExternalInput").ap()
    ox_d = nc.dram_tensor("out_x", [P, tpc], f32, kind="ExternalOutput").ap()
    od_d = nc.dram_tensor("out_dxdq", [P, tpc, Q], f32, kind="ExternalOutput").ap()

    with tile.TileContext(nc) as tc:
        with (
            tc.tile_pool(name="small", bufs=1) as sp,
            tc.tile_pool(name="data", bufs=data_bufs) as dp,
        ):
            xs = sp.tile([P, tpc], f32, tag="xs")
            nc.sync.dma_start(out=xs[:], in_=x_d[:])
            sg = sp.tile([P, tpc], f32, tag="sg")
            nc.scalar.activation(sg[:], xs[:], act.Sigmoid)
            # No Softplus ACT table on this toolchain; use
            # softplus(x) = x - ln(sigmoid(x)).
            ox = sp.tile([P, tpc], f32, tag="ox")
            nc.scalar.activation(ox[:], sg[:], act.Ln)
            nc.vector.tensor_sub(ox[:], xs[:], ox[:])
            nc.scalar.dma_start(out=ox_d[:], in_=ox[:])

            for c in range(nchunk):
                t = dp.tile([P, t_chunk * Q], f32, tag="t")
                nc.sync.dma_start(out=t[:], in_=dxdq_d[:, c * t_chunk : (c + 1) * t_chunk, :])
                for j in range(t_chunk):
                    r = c * t_chunk + j
                    nc.vector.tensor_scalar_mul(
                        t[:, j * Q : (j + 1) * Q],
                        t[:, j * Q : (j + 1) * Q],
                        sg[:, r : r + 1],
                    )
                nc.scalar.dma_start(
                    out=od_d[:, c * t_chunk : (c + 1) * t_chunk, :], in_=t[:]
                )
    nc.compile()
    return nc


def kernel(x, dxdq):
    from concourse.bass_utils import run_bass_kernel_spmd

    if "nc" not in _cache:
        _cache["nc"] = _build_nc()
    nc = _cache["nc"]

    x = np.ascontiguousarray(x, dtype=np.float32)
    dxdq = np.ascontiguousarray(dxdq, dtype=np.float32)
    xs = x.reshape(NCORES, P, TPC)
    ds = dxdq.reshape(NCORES, P, TPC, Q)
    in_maps = [{"x": xs[i], "dxdq": ds[i]} for i in range(NCORES)]

    res = run_bass_kernel_spmd(nc, in_maps, list(range(NCORES))).results

    bpc = B // NCORES
    out_x = np.concatenate(
        [np.asarray(res[i]["out_x"]).reshape(bpc, N) for i in range(NCORES)], axis=0
    )
    out_dxdq = np.concatenate(
        [np.asarray(res[i]["out_dxdq"]).reshape(bpc, N, Q) for i in range(NCORES)],
        axis=0,
    )
    return out_x, out_dxdq
